# revision 52
# baseline (speedup 1.0000x reference)
"""TF-IDF document model (histogram_binning) on 8 TRN2 NeuronCores.

Data-parallel over batch: 64 rows per core. Per row, the tf histogram over
vocab V=50257 is computed as a radix one-hot matmul on the PE:
v = hi*394 + lo, hi in [0,128), lo in [0,394).

Key structure (vs. the naive per-chunk one-hot kernel):
  - Host sorts each row's tokens by lo. Chunk c (sorted positions
    [128c,128c+128)) then covers a narrow static lo-window [Q[c], Q[c]+W[c]),
    so each accumulating matmul streams only ~100 output columns instead of
    394. (Windows are validated against the input; a data-derived build is
    used as fallback.)
  - All 8 hi one-hots of a row are built by ONE DVE tensor_tensor is_equal
    with a broadcast access pattern (in0 = hif row chunk broadcast along an
    h-major axis), hitting the 2x DVE mode: A_int[p, h*8+c] = (hif[p,c]==h).
    The matmul lhsT reads the per-chunk one-hot via a strided AP.
  - The per-token idf value (host gather idf[x], like the baseline's host
    divmod) rides the lo one-hot build as the tensor_scalar op1 multiplier,
    so no separate (B,V)-sized tf*idf multiply pass exists.
  - The lo one-hot builds are split between the DVE and the otherwise idle
    GPSIMD (Pool) engine. Pool takes the EARLY chunks (built while the DVE
    is busy with the row's hi one-hots) plus the narrow edge windows; the
    DVE takes late chunks 3-6 (4 of them on half the rows). The split is
    scheduler-tuned: build arrival order matches the sequential PSUM
    accumulation chain.
  - PSUM is cleared by a K=1 zero matmul, the 8 windowed matmuls accumulate,
    and the ACT engine's PSUM->SBUF copy applies the per-row 1/n scale and
    converts to fp16 (halving the output DMA). Host upcasts to fp32.
  - Head latency: iota/zero/one constants are generated on-device (Pool
    iota + memset) so no engine waits on a const DMA; the hif/lof/idf
    inputs ship as head-slice + remainder DMAs on two queues so row 0's
    builds start ~2.3us in. Output DMAs alternate sync/scalar queues; the
    final group is split into per-row DMAs (one drained by the DVE) to
    shorten the end-of-kernel chain.
"""
import numpy as np

import concourse.bacc as bacc
import concourse.mybir as mybir
from concourse import bass_utils
from concourse.tile import TileContext

B, S, V = 512, 1024, 50257
NC = 8
BL = B // NC          # 64 rows per core
HI, LO = 128, 394     # radix split: v = hi*LO + lo
VP = HI * LO          # 50432 padded vocab
CH = S // 128         # 8 sorted 128-token chunks per row
GROUP = 2             # rows per output DMA

# static lo-windows per sorted chunk (observed data bounds +-4; the host
# prep asserts every token falls inside its window, kernel() falls back to
# a data-derived build if violated)
QS = [0, 35, 78, 126, 176, 222, 275, 328]
WS = [66, 84, 92, 90, 92, 90, 84, 66]

_cache = {}


POOL_CHUNKS = (0, 1, 2, 3, 6, 7)   # built by ONE gpsimd local_scatter per row
DVE_CHUNKS = (4, 5)                # built by DVE tensor_scalar


def _scatter_offsets(ws):
    """Column offsets of each POOL_CHUNK inside the concatenated scatter
    tile, and the (even) total width."""
    offs = {}
    o = 0
    for c in POOL_CHUNKS:
        offs[c] = o
        o += ws[c]
    return offs, o + (o % 2)


def _build(repeat: int = 0, feat: str = "full", qs=None, ws=None):
    QS, WS = (qs or globals()["QS"]), (ws or globals()["WS"])
    WMAX = max(WS)
    nc = bacc.Bacc(
        "TRN2",
        target_bir_lowering=False,
        debug=False,
        enable_asserts=False,
        num_devices=NC,
    )
    ncols = BL * CH
    # data inputs only: hif (fp16, sync queue) and pk32 = lof ++ idfv (fp32,
    # scalar queue); all constants (iotas, zeros, ones) are generated
    # on-device at t=0 so no engine waits on a const DMA
    P32 = 2 * ncols
    HCOLS = 8 * CH
    HC16 = 4 * CH  # first 4 rows' hif, shipped in a tiny head DMA
    hifa_t = nc.dram_tensor("hif16a", [128, HC16], mybir.dt.float16, kind="ExternalInput")
    hifb_t = nc.dram_tensor("hif16b", [128, ncols - HC16], mybir.dt.float16, kind="ExternalInput")
    pk32a_t = nc.dram_tensor("pk32a", [128, 2 * HCOLS], mybir.dt.float32, kind="ExternalInput")
    pk32b_t = nc.dram_tensor("pk32b", [128, P32 - 2 * HCOLS], mybir.dt.float32, kind="ExternalInput")
    NIDX = len(POOL_CHUNKS)
    SCH = 8 * NIDX  # first 8 rows' scatter-pack slice (head DMA)
    sc16a_t = nc.dram_tensor("sc16a", [128, 2 * SCH], mybir.dt.int16, kind="ExternalInput")
    sc16b_t = nc.dram_tensor("sc16b", [128, 2 * (BL - 8) * NIDX], mybir.dt.int16, kind="ExternalInput")
    # transposed layout: out[p, r*LO+f] = row r, vocab p*LO+f (host unshuffles)
    out_t = nc.dram_tensor("out", [128, BL * LO], mybir.dt.float16, kind="ExternalOutput")
    ovg = out_t.ap().rearrange("p (g c) -> g p c", g=BL // GROUP)

    AF = mybir.ActivationFunctionType
    OP = mybir.AluOpType

    with TileContext(nc) as tc:
        with (
            tc.tile_pool(name="const", bufs=1) as cpool,
            tc.tile_pool(name="aall", bufs=10) as apool,
            tc.tile_pool(name="bt", bufs=64) as bpool,
            tc.tile_pool(name="tt", bufs=6) as tpool,
            tc.tile_pool(name="small", bufs=2) as spool,
            tc.tile_pool(name="ps", bufs=6, space="PSUM") as pspool,
            tc.tile_pool(name="ps2", bufs=1, space="PSUM") as ps2pool,
        ):
            # sync queue order: tiny hif head, tiny scatter-pack head, rests
            hif16 = cpool.tile([128, ncols], mybir.dt.float16, tag="hif16")
            nc.sync.dma_start(out=hif16[:, :HC16], in_=hifa_t.ap())
            sc16 = cpool.tile([128, 2 * BL * NIDX], mybir.dt.int16, tag="sc16")
            # tile layout: idxs rows0-7 | data rows0-7 | idxs rest | data rest
            nc.sync.dma_start(out=sc16[:, : 2 * SCH], in_=sc16a_t.ap())
            nc.sync.dma_start(out=hif16[:, HC16:], in_=hifb_t.ap())
            nc.sync.dma_start(out=sc16[:, 2 * SCH :], in_=sc16b_t.ap())
            pk32 = cpool.tile([128, P32], mybir.dt.float32, tag="pk32")
            nc.scalar.dma_start(out=pk32[:, : 2 * HCOLS], in_=pk32a_t.ap())
            nc.scalar.dma_start(out=pk32[:, 2 * HCOLS :], in_=pk32b_t.ap())
            soffs, SWID = _scatter_offsets(WS)

            def sc_idx(r):
                if r < 8:
                    return sc16[:, r * NIDX : (r + 1) * NIDX]
                o = 2 * SCH + (r - 8) * NIDX
                return sc16[:, o : o + NIDX]

            def sc_dat(r):
                if r < 8:
                    o = SCH + r * NIDX
                else:
                    o = 2 * SCH + (BL - 8) * NIDX + (r - 8) * NIDX
                return sc16[:, o : o + NIDX].bitcast(mybir.dt.float16)

            # on-device constants (no DMA deps): iotas on Pool, memsets on Pool
            ioc = cpool.tile([128, HI * CH + WMAX], mybir.dt.float16, tag="ioc")
            nc.gpsimd.iota(
                ioc[:, 0 : HI * CH],
                pattern=[[1, HI], [0, CH]],
                base=0,
                channel_multiplier=0,
                allow_small_or_imprecise_dtypes=True,
            )
            nc.gpsimd.iota(
                ioc[:, HI * CH : HI * CH + WMAX],
                pattern=[[1, WMAX]],
                base=0,
                channel_multiplier=0,
                allow_small_or_imprecise_dtypes=True,
            )
            z16 = cpool.tile([1, 128 + LO], mybir.dt.float16, tag="z16")
            nc.gpsimd.memset(z16[:], 0.0)
            ones32 = cpool.tile([1, 128], mybir.dt.float32, tag="ones32")
            nc.gpsimd.memset(ones32[:], 1.0)
            onesc32 = cpool.tile([128, 1], mybir.dt.float32, tag="onesc32")
            nc.gpsimd.memset(onesc32[:], 1.0)

            hif = hif16[:, :]
            iotar = ioc[:, 0 : HI * CH]
            iotaw = ioc[:, HI * CH : HI * CH + WMAX]

            NB = ncols - HCOLS

            def lof(col):
                if col < HCOLS:
                    return pk32[:, col : col + 1]
                o = 2 * HCOLS + (col - HCOLS)
                return pk32[:, o : o + 1]

            def idfv(col):
                if col < HCOLS:
                    return pk32[:, HCOLS + col : HCOLS + col + 1]
                o = 2 * HCOLS + NB + (col - HCOLS)
                return pk32[:, o : o + 1]

            idfvA = pk32[:, HCOLS : 2 * HCOLS]
            idfvB = pk32[:, 2 * HCOLS + NB : P32]
            onesc = onesc32[:, :]
            zcol = z16[:, 0:128]
            zrow = z16[:, 128 : 128 + LO]
            onesr = ones32[:, :]

            iotar3 = iotar.rearrange("p (h c) -> p h c", c=CH)

            def main_body(_iv=None):
                # --- per-row 1/n: n_r = sum_t idf[x[r,t]] ---
                n_ps = ps2pool.tile([1, ncols], mybir.dt.float32, tag="nps")
                nc.tensor.matmul(out=n_ps[:, :HCOLS], lhsT=onesc, rhs=idfvA, start=True, stop=True)
                nc.tensor.matmul(out=n_ps[:, HCOLS:], lhsT=onesc, rhs=idfvB, start=True, stop=True)
                nsum = spool.tile([1, BL], mybir.dt.float32, tag="nsum")
                nc.vector.tensor_reduce(
                    out=nsum[:],
                    in_=n_ps[:].rearrange("p (r c) -> p r c", c=CH),
                    axis=mybir.AxisListType.X,
                    op=OP.add,
                )
                recip = spool.tile([1, BL], mybir.dt.float32, tag="recip")
                nc.vector.reciprocal(out=recip[:], in_=nsum[:])
                rb_ps = ps2pool.tile([128, BL], mybir.dt.float32, tag="rbps")
                nc.tensor.matmul(out=rb_ps[:], lhsT=onesr, rhs=recip[:], start=True, stop=True)
                rb = spool.tile([128, BL], mybir.dt.float32, tag="rb")
                nc.scalar.activation(out=rb[:], in_=rb_ps[:], func=AF.Copy, scale=1.0)

                ngroups = BL // GROUP
                for g in range(ngroups):
                    # the final group is split into per-row DMAs so the
                    # end-of-kernel drain->DMA chain is as short as possible
                    split_tail = g == ngroups - 1
                    if not split_tail:
                        Tg = tpool.tile([128, GROUP * LO], mybir.dt.float16, tag="Tg")
                    for rr in range(GROUP):
                        r = g * GROUP + rr
                        # fused hi one-hots: A[p, h*CH+c] = (hif[p, r*CH+c] == h)
                        Aall = apool.tile([128, HI * CH], mybir.dt.float16, tag="Aall")
                        hif_exp = hif[:, r * CH : (r + 1) * CH].unsqueeze(1).broadcast_to(
                            [128, HI, CH]
                        )
                        nc.vector.tensor_tensor(
                            out=Aall[:].rearrange("p (h c) -> p h c", c=CH),
                            in0=hif_exp,
                            in1=iotar3,
                            op=OP.is_equal,
                        )
                        Aall3 = Aall[:].rearrange("p (h c) -> p c h", c=CH)

                        C = pspool.tile([128, LO], mybir.dt.float32, tag="C")
                        nc.tensor.matmul(out=C[:], lhsT=zcol, rhs=zrow, start=True, stop=False)
                        # one GPSIMD local_scatter builds SIX chunks' B one-hots
                        # (idf values scattered at host-computed offsets)
                        Bs = bpool.tile([128, SWID], mybir.dt.float16, tag="BS")
                        nc.gpsimd.local_scatter(
                            out_ap=Bs[:],
                            data_ap=sc_dat(r),
                            idxs_ap=sc_idx(r),
                            channels=128,
                            num_elems=SWID,
                            num_idxs=NIDX,
                        )
                        for c in range(CH):
                            col = r * CH + c
                            if c in DVE_CHUNKS:
                                Bt = bpool.tile([128, WMAX], mybir.dt.float16, tag="B")
                                nc.vector.tensor_scalar(
                                    out=Bt[:, : WS[c]],
                                    in0=iotaw[:, : WS[c]],
                                    scalar1=lof(col),
                                    scalar2=idfv(col),
                                    op0=OP.is_equal,
                                    op1=OP.mult,
                                )
                                rhs = Bt[:, : WS[c]]
                            else:
                                rhs = Bs[:, soffs[c] : soffs[c] + WS[c]]
                            nc.tensor.matmul(
                                out=C[:, QS[c] : QS[c] + WS[c]],
                                lhsT=Aall3[:, c, :],
                                rhs=rhs,
                                start=False,
                                stop=(c == CH - 1),
                            )
                        if split_tail:
                            # drain the two final rows on DIFFERENT engines
                            # (ACT + DVE) and dispatch their DMAs on
                            # different queues, shortening the end chain
                            Tr = tpool.tile([128, LO], mybir.dt.float16, tag="Tr")
                            if rr == GROUP - 1:
                                nc.vector.tensor_scalar(
                                    out=Tr[:],
                                    in0=C[:],
                                    scalar1=rb[:, r : r + 1],
                                    scalar2=None,
                                    op0=OP.mult,
                                )
                            else:
                                nc.scalar.activation(
                                    out=Tr[:],
                                    in_=C[:],
                                    func=AF.Copy,
                                    scale=rb[:, r : r + 1],
                                )
                            if feat == "nodma":
                                nc.vector.tensor_copy(out=nsum[:, :1], in_=Tr[:1, :1])
                            else:
                                q = nc.sync if rr == GROUP - 1 else nc.scalar
                                q.dma_start(
                                    out=out_t.ap()[:, r * LO : (r + 1) * LO], in_=Tr[:]
                                )
                        else:
                            nc.scalar.activation(
                                out=Tg[:, rr * LO : (rr + 1) * LO],
                                in_=C[:],
                                func=AF.Copy,
                                scale=rb[:, r : r + 1],
                            )
                    if not split_tail:
                        if feat == "nodma":
                            nc.vector.tensor_copy(out=nsum[:, :1], in_=Tg[:1, :1])
                        else:
                            oq = (nc.sync, nc.scalar)[g % 2]
                            oq.dma_start(out=ovg[g], in_=Tg[:])

            if repeat:
                tc.For_i_unrolled(0, repeat, 1, main_body, max_unroll=1)
            else:
                main_body()
    nc.compile()
    return nc


def _get_nc():
    if "nc" not in _cache:
        _cache["nc"] = _build()
    return _cache["nc"]


def _fits(lo_s: np.ndarray, qs, ws) -> bool:
    lo_c = lo_s.reshape(B, CH, 128)
    qa = np.asarray(qs, dtype=np.int32)[None, :, None]
    wa = np.asarray(ws, dtype=np.int32)[None, :, None]
    return bool(((lo_c >= qa) & (lo_c < qa + wa)).all())


def _windows_from_data(lo_s: np.ndarray):
    """Data-derived safe windows (used only if the static ones don't fit)."""
    qs, ws = [], []
    lo_c = lo_s.reshape(B, CH, 128)
    for c in range(CH):
        lo_b = max(0, int(lo_c[:, c].min()) - 2)
        hi_b = min(LO, int(lo_c[:, c].max()) + 1 + 2)
        w = (hi_b - lo_b + 1) // 2 * 2
        if lo_b + w > LO:
            lo_b = LO - w
        qs.append(lo_b)
        ws.append(w)
    return qs, ws


def _host_inputs(x: np.ndarray, idf: np.ndarray, qs=None, ws=None):
    """Build per-core input maps from the full inputs."""
    qs, ws = (qs or QS), (ws or WS)
    wmax = max(ws)
    xi = np.asarray(x, dtype=np.int64).astype(np.int32)  # values < 2**31
    idf32 = np.asarray(idf, dtype=np.float32)
    hi_all = (xi // LO).astype(np.int32)
    lo_all = (xi % LO).astype(np.int32)

    # sort each row's tokens by lo so each 128-chunk falls in a narrow window
    order = np.argsort(lo_all, axis=1, kind="stable")
    hi_s = np.take_along_axis(hi_all, order, axis=1)
    lo_s = np.take_along_axis(lo_all, order, axis=1)
    xs = np.take_along_axis(xi, order, axis=1)
    idfv_s = idf32[xs]  # (B, S) fp32, host gather (index prep like hif/lof)

    # per-chunk window-local lo
    qa = np.asarray(qs, dtype=np.int32)
    wa = np.asarray(ws, dtype=np.int32)
    lo_c = lo_s.reshape(B, CH, 128) - qa[None, :, None]
    assert lo_c.min() >= 0 and (lo_c < wa[None, :, None]).all(), "lo window overflow"

    hif = hi_s.astype(np.float16)
    lof = lo_c.reshape(B, S).astype(np.float32)
    idfv = idfv_s.astype(np.float32)

    ncols = BL * CH
    in_maps = []
    for k in range(NC):
        # layout [128, BL*CH]: element [p, r*CH+c] = token (row r, sorted pos c*128+p)
        def lay(a):
            ac = a[k * BL : (k + 1) * BL]
            return np.ascontiguousarray(
                ac.reshape(BL, CH, 128).transpose(2, 0, 1).reshape(128, BL * CH)
            )
        HC = 8 * CH
        lof_l, idfv_l = lay(lof), lay(idfv)
        pk32a = np.concatenate([lof_l[:, :HC], idfv_l[:, :HC]], axis=1)
        pk32b = np.concatenate([lof_l[:, HC:], idfv_l[:, HC:]], axis=1)
        hif_l = lay(hif)
        # scatter pack: per (row, pool-chunk-slot): index into the
        # concatenated scatter tile (int16) and the idf value (fp16 bits)
        offs, _swid = _scatter_offsets(ws)
        NIDX = len(POOL_CHUNKS)
        lof_l, idfv_l = lay(lof), lay(idfv)
        idx16 = np.empty((128, BL * NIDX), dtype=np.int16)
        dat16 = np.empty((128, BL * NIDX), dtype=np.float16)
        for r in range(BL):
            for k, c in enumerate(POOL_CHUNKS):
                idx16[:, r * NIDX + k] = offs[c] + lof_l[:, r * CH + c].astype(np.int16)
                dat16[:, r * NIDX + k] = idfv_l[:, r * CH + c].astype(np.float16)
        SCH = 8 * NIDX
        d16 = dat16.view(np.int16)
        sc16a = np.concatenate([idx16[:, :SCH], d16[:, :SCH]], axis=1)
        sc16b = np.concatenate([idx16[:, SCH:], d16[:, SCH:]], axis=1)
        in_maps.append({"hif16a": hif_l[:, : 4 * CH].copy(), "hif16b": hif_l[:, 4 * CH :].copy(),
                        "pk32a": pk32a, "pk32b": pk32b, "sc16a": sc16a, "sc16b": sc16b})
    return in_maps


def kernel(x: np.ndarray, idf: np.ndarray) -> np.ndarray:
    # check the static windows against this input; fall back to data-derived
    # windows (fresh build) if they don't fit
    xi = np.asarray(x, dtype=np.int64).astype(np.int32)
    lo_s = np.sort((xi % LO).astype(np.int32), axis=1)
    if _fits(lo_s, QS, WS):
        nc = _get_nc()
        in_maps = _host_inputs(x, idf)
    else:
        qs, ws = _windows_from_data(lo_s)
        key = ("dyn", tuple(qs), tuple(ws))
        if key not in _cache:
            _cache[key] = _build(qs=qs, ws=ws)
        nc = _cache[key]
        in_maps = _host_inputs(x, idf, qs, ws)
    res = bass_utils.run_bass_kernel_spmd(nc, in_maps, core_ids=list(range(NC)))
    outs = []
    for r in res.results:
        a = r["out"].reshape(128, BL, LO).transpose(1, 0, 2).reshape(BL, VP)
        outs.append(a[:, :V].astype(np.float32))
    return np.concatenate(outs, axis=0)

